# revision 1
# baseline (speedup 1.0000x reference)
"""Trainium2 Bass kernel for a 4-layer attention transformer whose input is
one-hot tokens concat one-hot positions.

Algorithm (algebraically identical to the dense reference):
  X_0 = [T, I] with T = onehot(tokens) [n, v], I = eye(n).
  Each layer X <- softmax(X R X^T + causal) X keeps the factored form
  X_k = [P_k T, P_k] where P_k = A_k ... A_1 is lower-triangular [n, n].
  Per layer we only need:
    W   = R[tok] + R[v:]            (row gather, [n, d])
    encR^T = (P W)^T                (matmul, contracted over n)
    G^T = T^T @ encR^T[:v] + encR^T[v:]   (one-hot matmul = column gather)
    scores = (G^T)^T P^T + causal   ->  A = softmax -> P <- A P
  The final layer only needs the last row of X_4, and logits = last @ U^T.

Sharding: data-parallel over batch, 32/8 = 4 batch elements per core;
R stack and unembed weight replicated (pre-cast to fp16 on host; all
matmul accumulation is fp32 in PSUM, softmax path is fp32).
"""

import numpy as np

import concourse.bass as bass
import concourse.bacc as bacc
import concourse.mybir as mybir
import concourse.tile as tile
from concourse.bass import IndirectOffsetOnAxis
from concourse.bass_utils import run_bass_kernel_spmd
from concourse.masks import make_identity

P = 128
VOCAB, CTX, D, L = 512, 1024, 1536, 4
BPC = 4                    # batch elements per core
NCORES = 8
MT = CTX // P              # 8 m-tiles
VT = VOCAB // P            # 4 vocab tiles
DT = D // P                # 12 d-tiles
NEG = -1.0e9
F32 = mybir.dt.float32
I32 = mybir.dt.int32
ST = mybir.dt.float16      # storage dtype for matmul operands
NP_ST = np.float16
AX = mybir.AxisListType.X
ALU = mybir.AluOpType
AF = mybir.ActivationFunctionType


def _chunks(w, step=512):
    j0 = 0
    while j0 < w:
        wc = min(step, w - j0)
        yield j0, wc
        j0 += wc


def emit(ctx, tc, tok_d, R_d, ut_d, out_d):
    nc = tc.nc

    const = ctx.enter_context(tc.tile_pool(name="const", bufs=1))
    state = ctx.enter_context(tc.tile_pool(name="state", bufs=1))
    stream = ctx.enter_context(tc.tile_pool(name="stream", bufs=2))
    psmm = ctx.enter_context(tc.tile_pool(name="psmm", bufs=4, space="PSUM"))
    pssc = ctx.enter_context(tc.tile_pool(name="pssc", bufs=2, space="PSUM"))
    pstp = psmm

    # ---- constants ----
    ident = const.tile([P, P], ST)
    make_identity(nc, ident)
    masktile = const.tile([P, P], F32)       # -1e9 above diag, 0 on/below
    idiff_i = const.tile([P, P], I32)        # value = j - p
    nc.gpsimd.iota(idiff_i, pattern=[[1, P]], base=0, channel_multiplier=-1)
    idiff_f = const.tile([P, P], F32)
    nc.vector.tensor_copy(idiff_f, idiff_i)
    nc.vector.tensor_scalar(
        out=masktile, in0=idiff_f, scalar1=0.5, scalar2=NEG, op0=ALU.is_ge,
        op1=ALU.mult,
    )  # ((j - p) >= 0.5) * -1e9
    iota512_i = const.tile([P, VOCAB], I32)
    nc.gpsimd.iota(iota512_i, pattern=[[1, VOCAB]], base=0, channel_multiplier=0)
    iota512f = const.tile([P, VOCAB], F32)
    nc.vector.tensor_copy(iota512f, iota512_i)
    vtcol_i = const.tile([P, VT], I32)       # value = p + 128*vt
    nc.gpsimd.iota(vtcol_i, pattern=[[P, VT]], base=0, channel_multiplier=1)
    vtcolf = const.tile([P, VT], F32)
    nc.vector.tensor_copy(vtcolf, vtcol_i)
    lastcol = const.tile([P, DT, BPC], ST)
    # DRAM scratch for the encR^T vocab rows (token-gathered back into G^T);
    # standalone tensors (not a DRAM pool) because indirect DMA sources must
    # have offset 0.
    scratch = [
        nc.dram_tensor(f"ertscr{i}", [VOCAB, CTX], ST, kind="Internal").ap()
        for i in range(2)
    ]

    def l4_compute(W4, tokidx, TTt, Pcur, Qcur, b):
        # encR4 last row as a column: e4col[:, dt] = sum_m W4[m, dt] * P3[last, m]
        e4col = stream.tile([P, DT], ST, tag="e4col")
        for dt in range(DT):
            ps = psmm.tile([P, 1], F32, tag="mm")
            for mt in range(MT):
                nc.tensor.matmul(
                    ps,
                    lhsT=W4[:, mt, dt * P : (dt + 1) * P],
                    rhs=Qcur[:, mt, CTX - 1 : CTX],
                    start=(mt == 0), stop=(mt == MT - 1),
                )
            nc.vector.tensor_copy(e4col[:, dt : dt + 1], ps)
        # G4 column: G4[m] = e4[tok[m]] + e4[v + m]
        g4col = stream.tile([P, MT], ST, tag="g4col")
        for mt in range(MT):
            ps = psmm.tile([P, 1], F32, tag="mm")
            for vt in range(VT):
                nc.tensor.matmul(
                    ps,
                    lhsT=TTt[:, vt, mt * P : (mt + 1) * P],
                    rhs=e4col[:, vt : vt + 1],
                    start=(vt == 0), stop=(vt == VT - 1),
                )
            nc.vector.tensor_add(
                g4col[:, mt : mt + 1], ps, e4col[:, VT + mt : VT + mt + 1]
            )
        # scores4 last row (no mask: row n-1 sees everything)
        S4 = stream.tile([1, CTX], F32, tag="E", name="S4", bufs=3)
        for j0, wc in _chunks(CTX):
            ps = psmm.tile([1, 512], F32, tag="mm")
            for mt in range(MT):
                nc.tensor.matmul(
                    ps[:, :wc],
                    lhsT=g4col[:, mt : mt + 1],
                    rhs=Qcur[:, mt, j0 : j0 + wc],
                    start=(mt == 0), stop=(mt == MT - 1),
                )
            nc.vector.tensor_copy(S4[:, j0 : j0 + wc], ps[:, :wc])
        negmax4 = stream.tile([1, 1], F32, tag="negmax4")
        nc.vector.reduce_max(negmax4, S4, axis=AX, negate=True)
        E4 = stream.tile([1, CTX], F32, tag="E", name="E4", bufs=3)
        ssum4 = stream.tile([1, 1], F32, tag="ssum4")
        nc.scalar.activation(E4, S4, AF.Exp, bias=negmax4, accum_out=ssum4)
        rsum4 = stream.tile([1, 1], F32, tag="rsum4")
        nc.vector.reciprocal(rsum4, ssum4)
        a4 = stream.tile([1, CTX], ST, tag="Ast", name="a4")
        nc.vector.tensor_scalar_mul(a4, E4, rsum4)
        a4c = stream.tile([P, MT], ST, tag="ep", name="a4c")
        for mt in range(MT):
            tp = pstp.tile([P, P], ST, tag="mm", name="tp")
            nc.tensor.transpose(tp[:, :1], a4[:, mt * P : (mt + 1) * P], ident[:1, :1])
            nc.vector.tensor_copy(a4c[:, mt : mt + 1], tp[:, :1])
        # p4 = a4 @ P3 (row), then transpose to a column
        p4 = stream.tile([1, CTX], ST, tag="Ast", name="p4")
        for j0, wc in _chunks(CTX):
            ps = psmm.tile([1, 512], F32, tag="mm")
            for mt in range(MT):
                nc.tensor.matmul(
                    ps[:, :wc],
                    lhsT=a4c[:, mt : mt + 1],
                    rhs=Pcur[:, mt, j0 : j0 + wc],
                    start=(mt == 0), stop=(mt == MT - 1),
                )
            nc.vector.tensor_copy(p4[:, j0 : j0 + wc], ps[:, :wc])
        p4c = stream.tile([P, MT], ST, tag="ep", name="p4c")
        for mt in range(MT):
            tp = pstp.tile([P, P], ST, tag="mm", name="tp")
            nc.tensor.transpose(tp[:, :1], p4[:, mt * P : (mt + 1) * P], ident[:1, :1])
            nc.vector.tensor_copy(p4c[:, mt : mt + 1], tp[:, :1])
        # last = [T^T-scatter of p4 (vocab), p4 (pos)] as a d-column
        Tt = state.tile([P, MT, VOCAB], ST, tag="GT", bufs=2)   # reuse GT slot
        for mt in range(MT):
            nc.vector.tensor_tensor(
                out=Tt[:, mt], in0=tokidx[:, mt : mt + 1].to_broadcast([P, VOCAB]),
                in1=iota512f, op=ALU.is_equal,
            )
        for vt in range(VT):
            ps = psmm.tile([P, 1], F32, tag="mm")
            for mt in range(MT):
                nc.tensor.matmul(
                    ps,
                    lhsT=Tt[:, mt, vt * P : (vt + 1) * P],
                    rhs=p4c[:, mt : mt + 1],
                    start=(mt == 0), stop=(mt == MT - 1),
                )
            nc.vector.tensor_copy(lastcol[:, vt, b : b + 1], ps)
        nc.vector.tensor_copy(lastcol[:, VT:, b], p4c)

    pending = None
    for b in range(BPC):
        # ---- token prep ----
        tokidx = stream.tile([P, MT], I32, tag="tokidx")
        nc.sync.dma_start(out=tokidx, in_=tok_d[b].rearrange("(t p) -> p t", p=P))
        tokb_i = stream.tile([P, CTX], I32, tag="tokbi", bufs=1)
        nc.sync.dma_start(out=tokb_i, in_=tok_d[b : b + 1, :].to_broadcast([P, CTX]))
        TTt = state.tile([P, VT, CTX], ST, tag="TT", bufs=2)   # T^T one-hot: [v, m]
        for vt in range(VT):
            nc.vector.tensor_tensor(
                out=TTt[:, vt], in0=tokb_i,
                in1=vtcolf[:, vt : vt + 1].to_broadcast([P, CTX]),
                op=ALU.is_equal,
            )

        Pcur = None   # [P, MT, CTX] fp16, lower-triangular P_k
        Qcur = None   # its transpose

        for k in range(3):
            # ---- W = R[k][tok] + R[k][v:]  ([n, d] rows in m-tiles) ----
            W = state.tile([P, MT, D], ST, tag="W", bufs=2)
            for mt in range(MT):
                nc.sync.dma_start(
                    out=W[:, mt], in_=R_d[k][VOCAB + mt * P : VOCAB + (mt + 1) * P, :]
                )
            for mt in range(MT):
                nc.gpsimd.indirect_dma_start(
                    out=W[:, mt], out_offset=None, in_=R_d[k],
                    in_offset=IndirectOffsetOnAxis(ap=tokidx[:, mt : mt + 1], axis=0),
                    compute_op=ALU.add,
                )

            if k == 0:
                # encR^T = W^T; only the vocab part is needed as matmul lhsT.
                eRTv = state.tile([P, VT, CTX], ST, tag="eRTv")  # [v, i]
                for mt in range(MT):
                    for vt in range(VT):
                        tp = pstp.tile([P, P], ST, tag="mm", name="tp")
                        nc.tensor.transpose(tp, W[:, mt, vt * P : (vt + 1) * P], ident)
                        nc.scalar.copy(eRTv[:, vt, mt * P : (mt + 1) * P], tp)
                GT = None
                if pending is not None:
                    l4_compute(**pending)
                    pending = None
            else:
                # encR^T[d, i] = sum_m W[m, d] Q[m, i].  Pos rows land directly
                # in G^T; vocab rows are staged to DRAM and token-gathered back
                # on top of G^T via DMA (cce add) instead of a one-hot matmul.
                scr = scratch[(b * 3 + k) % 2]
                GT = state.tile([P, MT, CTX], ST, tag="GT", name="GT", bufs=2)
                for dt in range(DT):
                    st = None
                    if dt < VT:
                        st = stream.tile([P, CTX], ST, tag="ep", name="st")
                    for j0, wc in _chunks(CTX):
                        kmax = (j0 + wc + P - 1) // P
                        ps = psmm.tile([P, 512], F32, tag="mm")
                        for mt in range(kmax):
                            nc.tensor.matmul(
                                ps[:, :wc],
                                lhsT=W[:, mt, dt * P : (dt + 1) * P],
                                rhs=Qcur[:, mt, j0 : j0 + wc],
                                start=(mt == 0), stop=(mt == kmax - 1),
                            )
                        if dt < VT:
                            nc.scalar.copy(st[:, j0 : j0 + wc], ps[:, :wc])
                        else:
                            nc.scalar.copy(
                                GT[:, dt - VT, j0 : j0 + wc], ps[:, :wc]
                            )
                    if dt < VT:
                        nc.sync.dma_start(
                            out=scr[dt * P : (dt + 1) * P, :], in_=st
                        )
                for mt in range(MT):
                    nc.gpsimd.indirect_dma_start(
                        out=GT[:, mt], out_offset=None, in_=scr,
                        in_offset=IndirectOffsetOnAxis(
                            ap=tokidx[:, mt : mt + 1], axis=0
                        ),
                        compute_op=ALU.add,
                    )

            # ---- scores -> softmax -> A^T; P_new = A P; Q_new = P_new^T ----
            Pnew = state.tile([P, MT, CTX], ST, tag="P", bufs=2)
            Qnew = state.tile([P, MT, CTX], ST, tag="Q")
            ATt = None
            if k > 0:
                ATt = state.tile([P, MT, CTX], ST, tag="AT", name="ATt")
            for it in range(MT):
                w = (it + 1) * P
                # scores accumulate in PSUM ([P, w] across <=2 banks); the
                # causal mask is added pre-exp, so exp(-1e9) = 0 exactly and
                # no max-subtraction is needed (|scores| << 1 here).
                psc = pssc.tile([P, CTX], F32, tag="sc")
                for j0, wc in _chunks(w):
                    if k == 0:
                        # scores = G1 directly (P_0 = I): G1 = eRTv^T-matmul + pos
                        for vt in range(VT):
                            nc.tensor.matmul(
                                psc[:, j0 : j0 + wc],
                                lhsT=eRTv[:, vt, it * P : (it + 1) * P],
                                rhs=TTt[:, vt, j0 : j0 + wc],
                                start=(vt == 0), stop=(vt == VT - 1),
                            )
                        nc.vector.tensor_add(
                            psc[:, j0 : j0 + wc], psc[:, j0 : j0 + wc],
                            W[:, it, VOCAB + j0 : VOCAB + j0 + wc],
                        )
                    else:
                        kmax = (j0 + wc + P - 1) // P
                        for mt in range(kmax):
                            nc.tensor.matmul(
                                psc[:, j0 : j0 + wc],
                                lhsT=GT[:, mt, it * P : (it + 1) * P],
                                rhs=Qcur[:, mt, j0 : j0 + wc],
                                start=(mt == 0), stop=(mt == kmax - 1),
                            )
                nc.vector.tensor_add(psc[:, w - P : w], psc[:, w - P : w], masktile)
                E = stream.tile([P, CTX], F32, tag="E", bufs=3)
                ssum = stream.tile([P, 1], F32, tag="ssum")
                nc.scalar.activation(E[:, :w], psc[:, :w], AF.Exp, accum_out=ssum)
                rsum = stream.tile([P, 1], F32, tag="rsum")
                nc.vector.reciprocal(rsum, ssum)
                if k == 0:
                    Adst = Pnew[:, it, :]
                else:
                    Ast = stream.tile([P, CTX], ST, tag="Ast")
                    Adst = Ast
                nc.vector.tensor_scalar_mul(Adst[:, :w], E[:, :w], rsum)
                if w < CTX:
                    nc.vector.memset(Adst[:, w:], 0.0)
                if k > 0:
                    for jt in range(it + 1):
                        tp = pstp.tile([P, P], ST, tag="mm", name="tp")
                        nc.tensor.transpose(
                            tp, Ast[:, jt * P : (jt + 1) * P], ident
                        )
                        nc.scalar.copy(ATt[:, jt, it * P : (it + 1) * P], tp)

            if k > 0:
                for it2 in range(MT):
                    w2 = (it2 + 1) * P
                    for j0, wc in _chunks(w2):
                        ps = psmm.tile([P, 512], F32, tag="mm")
                        mts = list(range(j0 // P, it2 + 1))
                        for mi, mt in enumerate(mts):
                            nc.tensor.matmul(
                                ps[:, :wc],
                                lhsT=ATt[:, mt, it2 * P : (it2 + 1) * P],
                                rhs=Pcur[:, mt, j0 : j0 + wc],
                                start=(mi == 0), stop=(mi == len(mts) - 1),
                            )
                        nc.vector.tensor_copy(Pnew[:, it2, j0 : j0 + wc], ps[:, :wc])
                    if w2 < CTX:
                        nc.vector.memset(Pnew[:, it2, w2:], 0.0)

            # Q_new = P_new^T (upper-triangular); zero the never-written part.
            for mt in range(1, MT):
                nc.vector.memset(Qnew[:, mt, : mt * P], 0.0)
            for it in range(MT):
                for jt in range(it + 1):
                    tp = pstp.tile([P, P], ST, tag="mm", name="tp")
                    nc.tensor.transpose(
                        tp, Pnew[:, it, jt * P : (jt + 1) * P], ident
                    )
                    nc.vector.tensor_copy(Qnew[:, jt, it * P : (it + 1) * P], tp)
            Pcur, Qcur = Pnew, Qnew

        # ---- layer 4 inputs: gather W4 now; its compute is emitted during
        # the next batch's iteration so the serial last-row chain overlaps
        # that batch's first layer ----
        W4 = state.tile([P, MT, D], ST, tag="W", bufs=2, name="W4")
        for mt in range(MT):
            nc.sync.dma_start(
                out=W4[:, mt], in_=R_d[3][VOCAB + mt * P : VOCAB + (mt + 1) * P, :]
            )
        for mt in range(MT):
            nc.gpsimd.indirect_dma_start(
                out=W4[:, mt], out_offset=None, in_=R_d[3],
                in_offset=IndirectOffsetOnAxis(ap=tokidx[:, mt : mt + 1], axis=0),
                compute_op=ALU.add,
            )
        pending = dict(W4=W4, tokidx=tokidx, TTt=TTt, Pcur=Pcur, Qcur=Qcur, b=b)

    l4_compute(**pending)
    pending = None

    # ---- unembed: logits[b, :] = last_b @ U^T ----
    uts = state.tile([P, DT, VOCAB], ST, tag="GT", name="uts", bufs=2)
    nc.sync.dma_start(out=uts, in_=ut_d.rearrange("(dt p) v -> p dt v", p=P))
    outs = const.tile([BPC, VOCAB], F32)
    for j0, wc in _chunks(VOCAB):
        ps = psmm.tile([BPC, 512], F32, tag="mm")
        for dt in range(DT):
            nc.tensor.matmul(
                ps[:, :wc],
                lhsT=lastcol[:, dt, :],
                rhs=uts[:, dt, j0 : j0 + wc],
                start=(dt == 0), stop=(dt == DT - 1),
            )
        nc.vector.tensor_copy(outs[:, j0 : j0 + wc], ps[:, :wc])
    nc.sync.dma_start(out=out_d, in_=outs)


def build_program():
    nc = bacc.Bacc("TRN2", debug=False, num_devices=NCORES, num_swdge_queues=4)
    tok_d = nc.dram_tensor("tok", [BPC, CTX], I32, kind="ExternalInput").ap()
    R_d = [
        nc.dram_tensor(f"r{k}", [D, D], ST, kind="ExternalInput").ap()
        for k in range(L)
    ]
    ut_d = nc.dram_tensor("ut", [D, VOCAB], ST, kind="ExternalInput").ap()
    out_d = nc.dram_tensor("logits", [BPC, VOCAB], F32, kind="ExternalOutput").ap()
    from contextlib import ExitStack

    with tile.TileContext(nc) as tc:
        with ExitStack() as ctx:
            emit(ctx, tc, tok_d, R_d, ut_d, out_d)
    nc.compile()
    return nc


def make_in_maps(token_ids, R_stack, U):
    tok = np.asarray(token_ids).astype(np.int32).reshape(NCORES, BPC, CTX)
    R16 = [np.ascontiguousarray(np.asarray(R_stack[k]).astype(NP_ST)) for k in range(L)]
    ut16 = np.ascontiguousarray(np.asarray(U).astype(NP_ST).T)
    in_maps = []
    for c in range(NCORES):
        m = {"tok": np.ascontiguousarray(tok[c]), "ut": ut16}
        for k in range(L):
            m[f"r{k}"] = R16[k]
        in_maps.append(m)
    return in_maps


_cached_nc = None


def kernel(token_ids, R_stack, U, _want_time=False, _trace=False):
    global _cached_nc
    if _cached_nc is None:
        _cached_nc = build_program()
    in_maps = make_in_maps(token_ids, R_stack, U)
    res = run_bass_kernel_spmd(
        _cached_nc, in_maps, core_ids=list(range(NCORES)), trace=_trace
    )
    logits = np.concatenate([res.results[c]["logits"] for c in range(NCORES)], axis=0)
    if _want_time:
        return logits.astype(np.float32), res.exec_time_ns
    return logits.astype(np.float32)

